# revision 14
# baseline (speedup 1.0000x reference)
"""Trainium2 Bass kernel for nn_BidirectionalTrustModel.

Problem: T=64 steps of per-sequence running elementwise min/max over capability
vectors gathered from a tiny [C=32, 6] obsMatrix, then trust[b] = all_i
(required[b,i] <= mean[b,i]).

Algorithm (same bitmask algebra as validated previously): host precomputes
    W[p][l]   = bits_i[ M[i,l] < M[i,p] ]   (column l breaks requirement p at row i)
    W0[p]     = bits_i[ M[i,p] <= 0 ]
Per (t,b) let w = W[p_b][id_t].  With A = "OR of w over failure steps" and
V = "AND over success-step candidates ~(w | failures-after)", the pair (A, V)
composes associatively:   A = A_L | A_R ;  V = V_R & (V_L | A_R)
so the t-scan is a log-depth bitwise tree.  trust[b] = ((V & ~W0[p_b]) == 0).

v4 layout/engine changes vs v1 (86us):
 - partition dim = (t-half, 64 b-groups) so every DMA has >=512B contiguous
   runs (ids previously paid the sub-512B 2x DMA penalty)
 - per-(t,b) select runs as copy_predicated chain on DVE only (init copy +
   5 cp); masks split ACT (3, Square/Relu) / DVE (2, tensor_scalar)
 - gates A0=w&-p0, V0=w|(p1-1) as single scalar_tensor_tensor ops on GpSimd
 - tree levels 1-3 on GpSimd, cross-t-half level + cross-chunk + final on DVE
 - per-b tables (Wrow, ~W0) built from a replicated 31-word constant input
   via short cp chains instead of dozens of memset constants
"""
import sys

for _p in ("/opt/trn_rl_repo", "/root/.axon_site/_ro/trn_rl_repo"):
    if _p not in sys.path:
        sys.path.append(_p)

import numpy as np

from concourse import bass, mybir
from concourse.alu_op_type import AluOpType
from concourse.bass_utils import run_bass_kernel_spmd
from concourse.tile import TileContext
from concourse.vector_clock import ScopedClock, VectorClock


class SplitDrainTileContext(TileContext):
    """TileContext whose kernel-tail drain is split into a chain of drains,
    one semaphore wait each — walrus's DIRECT2D codegen rejects drains
    carrying more than a few sync waits ("Too many sync wait commands")."""

    def _drain_and_barrier(self, tick_clock, wait_clock):
        gc = tick_clock.global_clock
        n = len(gc)
        nonzero = [p for p in range(n) if gc[p] > 0]
        for p in nonzero:
            vc = VectorClock([gc[q] if q == p else 0 for q in range(n)])
            d = self.nc.sync.drain()
            wait_clock.add_sem_waits(d.ins, ScopedClock({None: vc}))
        self.nc.all_engine_barrier()
        assert self.sems is not None
        popped = self.nc._tile_sem_poison_stack.pop()
        assert popped is self._sem_poison
        self.nc.clear_and_free_semaphores(list(self.sems.allocated().values()))
        self.nc.all_engine_barrier()


def split_multi_waits(nc):
    """walrus codegen supports only ONE semaphore wait per instruction;
    move extra waits onto injected same-engine no-ops."""
    import bass_rust

    si_cls = None
    counter = [0]
    for fn in nc.m.functions:
        for bb in fn.blocks:
            insts = list(bb.instructions)
            out = []
            changed = False
            for inst in insts:
                si = getattr(inst, "sync_info", None)
                if si is not None and len(si.on_wait) > 1:
                    waits = list(si.on_wait)
                    if si_cls is None:
                        si_cls = type(si)
                    for wt in waits[:-1]:
                        counter[0] += 1
                        nop = bass_rust.InstNoOp(
                            name=f"waitsplit-{counter[0]}", ins=[], outs=[]
                        )
                        nop.engine = inst.engine
                        nop.sync_info = si_cls(on_wait=[wt], on_update=[])
                        out.append(nop)
                    inst.sync_info = si_cls(
                        on_wait=[waits[-1]], on_update=list(si.on_update)
                    )
                    changed = True
                out.append(inst)
            if changed:
                try:
                    bb.instructions[:] = out
                except TypeError:
                    bb.instructions = out
    return counter[0]


T = 64
B = 65536
C = 32
NT = 6
NCORES = 8
P = 128
G = 128         # b-groups on partitions
NQ = 64         # b's per group (bs // G)
TCC = 32        # t's per chunk
TH = TCC        # t's per partition (all of the chunk)
NCH = T // TCC
DMA_LOAD_ENGINE = "sync"
DMA_STORE_ENGINE = "gpsimd"


def host_tables(M: np.ndarray):
    """W[p][l], negW0[p] as int64 bit patterns from obsMatrix [C, NT]."""
    assert M.shape == (C, NT)
    assert (M >= 0).all(), "algorithm assumes non-negative obsMatrix"
    Mi = M.astype(np.float32)
    less = Mi[:, :, None] < Mi[:, None, :]  # [i, l, p]
    pw = (1 << np.arange(C, dtype=np.int64))[:, None, None]
    W = (less * pw).sum(axis=0).T.astype(np.uint32)  # [p, l]
    w0 = ((Mi <= 0.0) * pw[:, :, 0]).sum(axis=0).astype(np.uint32)  # [p]
    negW0 = (~w0).astype(np.uint32)
    return W.astype(np.int64), negW0.astype(np.int64)


def _i32(v):
    v = int(v) & 0xFFFFFFFF
    return v - (1 << 32) if v >= (1 << 31) else v


# wtab column layout: cols (p-1)*5 + (k-1) = W[p][k] for p,k in 1..5;
# cols 25..29 = negW0[1..5]; col 30 = -1.
NWTAB = 31


def host_wtab(W, negW0) -> np.ndarray:
    row = np.zeros(NWTAB, dtype=np.int64)
    for p in range(1, NT):
        for k in range(1, NT):
            row[(p - 1) * 5 + (k - 1)] = W[p][k]
        row[25 + (p - 1)] = negW0[p]
    row[30] = 0xFFFFFFFF
    r32 = np.array([_i32(v) for v in row], dtype=np.int32)
    return np.tile(r32[None, :], (P, 1))


def build_nc(W, negW0, bs):
    assert bs == G * NQ
    i32 = mybir.dt.int32
    f32 = mybir.dt.float32

    nc = bass.Bass()
    dma_load = getattr(nc, DMA_LOAD_ENGINE)
    dma_store = getattr(nc, DMA_STORE_ENGINE)
    perf = nc.declare_dram_parameter("perf", [T, bs, 2], i32, isOutput=False)
    ids = nc.declare_dram_parameter("ids", [T, bs, 1], i32, isOutput=False)
    pred = nc.declare_dram_parameter("pred", [bs, 1], i32, isOutput=False)
    wtab = nc.declare_dram_parameter("wtab", [P, NWTAB], i32, isOutput=False)
    outp = nc.declare_dram_parameter("trust", [bs, 1], f32, isOutput=True)

    vec, gp, act = nc.vector, nc.gpsimd, nc.scalar
    AF = mybir.ActivationFunctionType

    with SplitDrainTileContext(nc) as tc:
        with tc.tile_pool(name="pers", bufs=1) as pers, \
             tc.tile_pool(name="dmain", bufs=NCH) as dmain, \
             tc.tile_pool(name="pool", bufs=2) as pool, \
             tc.tile_pool(name="actout", bufs=2) as actout, \
             tc.tile_pool(name="tree", bufs=1) as tree:
            # ---------------- per-core prep ----------------
            wt = pers.tile([P, NWTAB], i32, tag="wtab")
            dma_load.dma_start(out=wt[:, :], in_=wtab.rearrange("p n -> p n"))

            predt = pers.tile([P, NQ], i32, tag="predt")
            dma_load.dma_start(
                out=predt[:, :],
                in_=pred.rearrange("(g q) one -> g (q one)", g=G),
            )

            # presence masks p_b == p
            cpP = {}
            for p in range(1, NT):
                m = pers.tile([P, NQ], i32, tag=f"cpP{p}")
                vec.tensor_scalar(m[:, :], predt[:, :], p, None, AluOpType.is_equal)
                cpP[p] = m

            # Wfull[:, k-1, :] = W[p_b][k]  (k = 1..5)
            wfull = pers.tile([P, 5, NQ], i32, tag="wfull")
            vec.tensor_copy(
                wfull[:, :, :],
                wt[:, 0:5][:, :, None].broadcast_to([P, 5, NQ]),
            )
            for p in range(2, NT):
                vec.copy_predicated(
                    wfull[:, :, :],
                    cpP[p][:, None, :].broadcast_to([P, 5, NQ]),
                    wt[:, (p - 1) * 5 : (p - 1) * 5 + 5][:, :, None].broadcast_to(
                        [P, 5, NQ]
                    ),
                )

            # nw0[b] = ~W0[p_b]
            nw0 = pers.tile([P, NQ], i32, tag="nw0")
            vec.memset(nw0[:, :], 0)
            for p in range(1, NT):
                vec.copy_predicated(
                    nw0[:, :],
                    cpP[p][:, :],
                    wt[:, 25 + p - 1 : 26 + p - 1].broadcast_to([P, NQ]),
                )

            negone = wt[:, 30:31]  # [P, 1]

            # f32 bias tiles for ACT Square (bias must be an AP for non-Copy)
            actb = {}
            for k in (0, 4, 5):
                bt = pers.tile([P, 1], f32, tag=f"actb{k}")
                vec.memset(bt[:, :], float(-k))
                actb[k] = bt

            # ---------------- chunks over t ----------------
            states = None
            for ch in range(NCH):
                t0 = ch * TCC
                perf_t = dmain.tile([P, TH, NQ, 2], i32, tag="perf")
                ids_t = dmain.tile([P, TH, NQ], i32, tag="ids")
                dma_load.dma_start(
                    out=perf_t[:, :, :, :],
                    in_=perf[t0 : t0 + TCC].rearrange(
                        "th (g q) c -> g th q c", g=G
                    ),
                )
                dma_load.dma_start(
                    out=ids_t[:, :, :],
                    in_=ids[t0 : t0 + TCC].rearrange(
                        "th (g q) one -> g th (q one)", g=G
                    ),
                )

                # masks: m0, m4, m5 on ACT (Square + Relu); m2, m3 on DVE
                ids16 = actout.tile([P, TH, NQ], mybir.dt.int16, tag="ids16")
                act.activation(
                    ids16[:, :, :], ids_t[:, :, :], AF.Copy, bias=0.0, scale=1.0
                )
                msk = {}
                for k in (0, 4):
                    sq = actout.tile([P, TH, NQ], mybir.dt.int16, tag="sq")
                    mk = actout.tile([P, TH, NQ], mybir.dt.int16, tag=f"m{k}")
                    act.activation(
                        sq[:, :, :], ids_t[:, :, :], AF.Square,
                        bias=actb[k][:, :], scale=1.0,
                    )
                    act.activation(
                        mk[:, :, :], sq[:, :, :], AF.Relu, bias=1.0, scale=-1.0
                    )
                    msk[k] = mk
                for k in (2, 3, 5):
                    mk = actout.tile([P, TH, NQ], mybir.dt.int16, tag=f"m{k}")
                    vec.tensor_scalar(
                        mk[:, :, :], ids16[:, :, :], k, None, AluOpType.is_equal
                    )
                    msk[k] = mk

                # select w = Wrow[id]; default (id==1) seeded by init copy
                w = pool.tile([P, TH, NQ], i32, tag="w")
                act.activation(
                    w[:, :, :],
                    wfull[:, 0, :][:, None, :].broadcast_to([P, TH, NQ]),
                    AF.Copy, bias=0.0, scale=1.0,
                )
                vec.copy_predicated(
                    w[:, :, :], msk[0][:, :, :],
                    negone[:, :, None].broadcast_to([P, TH, NQ]),
                )
                for k in (2, 3, 4, 5):
                    vec.copy_predicated(
                        w[:, :, :], msk[k][:, :, :],
                        wfull[:, k - 1, :][:, None, :].broadcast_to([P, TH, NQ]),
                    )

                # gates: A0 = w & -p0 == w*p0 (p0 in {0,1}, Pool int mult);
                # V0 = w | (p1-1) with s1 = p1-1 from ACT (bitwise-or is DVE-only)
                A0 = pool.tile([P, TH, NQ], i32, tag="A0")
                vec.scalar_tensor_tensor(
                    A0[:, :, :], w[:, :, :], 1, perf_t[:, :, :, 0],
                    AluOpType.mult, AluOpType.mult,
                )
                s1 = actout.tile([P, TH, NQ], i32, tag="s1")
                act.activation(
                    s1[:, :, :], perf_t[:, :, :, 1], AF.Copy, bias=-1.0, scale=1.0
                )
                V0 = pool.tile([P, TH, NQ], i32, tag="V0")
                vec.tensor_tensor(
                    V0[:, :, :], w[:, :, :], s1[:, :, :], AluOpType.bitwise_or
                )

                # in-partition tree over th: 8 -> 4 -> 2 -> 1
                A, V = A0, V0
                nt = TH
                lvl = 0
                while nt > 1:
                    nt //= 2
                    lvl += 1
                    eng = vec
                    An = tree.tile([P, nt, NQ], i32, tag=f"A{lvl}")
                    Vn = tree.tile([P, nt, NQ], i32, tag=f"V{lvl}")
                    AL, AR = A[:, 0::2, :], A[:, 1::2, :]
                    VL, VR = V[:, 0::2, :], V[:, 1::2, :]
                    eng.tensor_tensor(Vn[:, :, :], VL, AR, AluOpType.bitwise_or)
                    eng.tensor_tensor(
                        Vn[:, :, :], Vn[:, :, :], VR, AluOpType.bitwise_and
                    )
                    eng.tensor_tensor(An[:, :, :], AL, AR, AluOpType.bitwise_or)
                    A, V = An, Vn

                A4, V4 = A, V

                # fold into running cross-chunk state (t order)
                if ch == 0:
                    states = [A4, V4]
                else:
                    Ar, Vr = states
                    Vn = tree.tile([P, 1, NQ], i32, tag=f"Vc{ch}")
                    vec.tensor_tensor(
                        Vn[:, :, :], Vr[:, :, :], A4[:, :, :], AluOpType.bitwise_or
                    )
                    vec.tensor_tensor(
                        Vn[:, :, :], Vn[:, :, :], V4[:, :, :], AluOpType.bitwise_and
                    )
                    if ch < NCH - 1:
                        An = tree.tile([P, 1, NQ], i32, tag=f"Ac{ch}")
                        vec.tensor_tensor(
                            An[:, :, :], Ar[:, :, :], A4[:, :, :],
                            AluOpType.bitwise_or,
                        )
                    else:
                        An = A4
                    states = [An, Vn]
            A, V = states

            # ---------------- finalize ----------------
            x = tree.tile([P, NQ], i32, tag="fin")
            vec.tensor_tensor(x[:, :], V[:, 0, :], nw0[:, :], AluOpType.bitwise_and)
            vec.tensor_scalar(x[:, :], x[:, :], 0, None, AluOpType.is_equal)
            of = tree.tile([P, NQ], f32, tag="of")
            vec.tensor_copy(of[:, :], x[:, :])
            dma_store.dma_start(
                out=outp.rearrange("(g q) one -> g (q one)", g=G), in_=of[:, :]
            )

    split_multi_waits(nc)
    return nc


_CACHE = {}


def _get_nc(key, W, negW0, bs):
    if key not in _CACHE:
        _CACHE[key] = build_nc(W, negW0, bs)
    return _CACHE[key]


def build_in_maps(perf, ids, pred, wtab_np, bs):
    in_maps = []
    for c in range(NCORES):
        sl = slice(c * bs, (c + 1) * bs)
        in_maps.append(
            {
                "perf": perf[:, sl, :],
                "ids": ids[:, sl, :],
                "pred": pred[sl, :],
                "wtab": wtab_np,
            }
        )
    return in_maps


def kernel(inptasksperf, tasksobsids, taskspredids, obsMatrix):
    perf = np.ascontiguousarray(np.asarray(inptasksperf, dtype=np.int32))
    ids = np.ascontiguousarray(np.asarray(tasksobsids, dtype=np.int32))
    pred = np.ascontiguousarray(np.asarray(taskspredids, dtype=np.int32))
    M = np.asarray(obsMatrix, dtype=np.float32)

    W, negW0 = host_tables(M)
    wtab_np = host_wtab(W, negW0)
    bs = B // NCORES
    key = (W.tobytes(), negW0.tobytes(), bs)
    nc = _get_nc(key, W, negW0, bs)

    in_maps = build_in_maps(perf, ids, pred, wtab_np, bs)
    res = run_bass_kernel_spmd(nc, in_maps, list(range(NCORES)))
    out = np.concatenate([res.results[c]["trust"] for c in range(NCORES)], axis=0)
    return out.astype(np.float32)


# revision 15
# speedup vs baseline: 1.0323x; 1.0323x over previous
"""Trainium2 Bass kernel for nn_BidirectionalTrustModel.

Problem: T=64 steps of per-sequence running elementwise min/max over capability
vectors gathered from a tiny [C=32, 6] obsMatrix, then trust[b] = all_i
(required[b,i] <= mean[b,i]).

Algorithm (same bitmask algebra as validated previously): host precomputes
    W[p][l]   = bits_i[ M[i,l] < M[i,p] ]   (column l breaks requirement p at row i)
    W0[p]     = bits_i[ M[i,p] <= 0 ]
Per (t,b) let w = W[p_b][id_t].  With A = "OR of w over failure steps" and
V = "AND over success-step candidates ~(w | failures-after)", the pair (A, V)
composes associatively:   A = A_L | A_R ;  V = V_R & (V_L | A_R)
so the t-scan is a log-depth bitwise tree.  trust[b] = ((V & ~W0[p_b]) == 0).

v4 layout/engine changes vs v1 (86us):
 - partition dim = (t-half, 64 b-groups) so every DMA has >=512B contiguous
   runs (ids previously paid the sub-512B 2x DMA penalty)
 - per-(t,b) select runs as copy_predicated chain on DVE only (init copy +
   5 cp); masks split ACT (3, Square/Relu) / DVE (2, tensor_scalar)
 - gates A0=w&-p0, V0=w|(p1-1) as single scalar_tensor_tensor ops on GpSimd
 - tree levels 1-3 on GpSimd, cross-t-half level + cross-chunk + final on DVE
 - per-b tables (Wrow, ~W0) built from a replicated 31-word constant input
   via short cp chains instead of dozens of memset constants
"""
import sys

for _p in ("/opt/trn_rl_repo", "/root/.axon_site/_ro/trn_rl_repo"):
    if _p not in sys.path:
        sys.path.append(_p)

import numpy as np

from concourse import bass, mybir
from concourse.alu_op_type import AluOpType
from concourse.bass_utils import run_bass_kernel_spmd
from concourse.tile import TileContext
from concourse.vector_clock import ScopedClock, VectorClock


class SplitDrainTileContext(TileContext):
    """TileContext whose kernel-tail drain is split into a chain of drains,
    one semaphore wait each — walrus's DIRECT2D codegen rejects drains
    carrying more than a few sync waits ("Too many sync wait commands")."""

    def _drain_and_barrier(self, tick_clock, wait_clock):
        gc = tick_clock.global_clock
        n = len(gc)
        nonzero = [p for p in range(n) if gc[p] > 0]
        for p in nonzero:
            vc = VectorClock([gc[q] if q == p else 0 for q in range(n)])
            d = self.nc.sync.drain()
            wait_clock.add_sem_waits(d.ins, ScopedClock({None: vc}))
        self.nc.all_engine_barrier()
        assert self.sems is not None
        popped = self.nc._tile_sem_poison_stack.pop()
        assert popped is self._sem_poison
        self.nc.clear_and_free_semaphores(list(self.sems.allocated().values()))
        self.nc.all_engine_barrier()


def split_multi_waits(nc):
    """walrus codegen supports only ONE semaphore wait per instruction;
    move extra waits onto injected same-engine no-ops."""
    import bass_rust

    si_cls = None
    counter = [0]
    for fn in nc.m.functions:
        for bb in fn.blocks:
            insts = list(bb.instructions)
            out = []
            changed = False
            for inst in insts:
                si = getattr(inst, "sync_info", None)
                if si is not None and len(si.on_wait) > 1:
                    waits = list(si.on_wait)
                    if si_cls is None:
                        si_cls = type(si)
                    for wt in waits[:-1]:
                        counter[0] += 1
                        nop = bass_rust.InstNoOp(
                            name=f"waitsplit-{counter[0]}", ins=[], outs=[]
                        )
                        nop.engine = inst.engine
                        nop.sync_info = si_cls(on_wait=[wt], on_update=[])
                        out.append(nop)
                    inst.sync_info = si_cls(
                        on_wait=[waits[-1]], on_update=list(si.on_update)
                    )
                    changed = True
                out.append(inst)
            if changed:
                try:
                    bb.instructions[:] = out
                except TypeError:
                    bb.instructions = out
    return counter[0]


T = 64
B = 65536
C = 32
NT = 6
NCORES = 8
P = 128
G = 128         # b-groups on partitions
NQ = 64         # b's per group (bs // G)
TCC = 16        # t's per chunk
TH = TCC        # t's per partition (all of the chunk)
NCH = T // TCC
DMA_LOAD_ENGINE = "sync"
DMA_STORE_ENGINE = "gpsimd"


def host_tables(M: np.ndarray):
    """W[p][l], negW0[p] as int64 bit patterns from obsMatrix [C, NT]."""
    assert M.shape == (C, NT)
    assert (M >= 0).all(), "algorithm assumes non-negative obsMatrix"
    Mi = M.astype(np.float32)
    less = Mi[:, :, None] < Mi[:, None, :]  # [i, l, p]
    pw = (1 << np.arange(C, dtype=np.int64))[:, None, None]
    W = (less * pw).sum(axis=0).T.astype(np.uint32)  # [p, l]
    w0 = ((Mi <= 0.0) * pw[:, :, 0]).sum(axis=0).astype(np.uint32)  # [p]
    negW0 = (~w0).astype(np.uint32)
    return W.astype(np.int64), negW0.astype(np.int64)


def _i32(v):
    v = int(v) & 0xFFFFFFFF
    return v - (1 << 32) if v >= (1 << 31) else v


# wtab column layout: cols (p-1)*5 + (k-1) = W[p][k] for p,k in 1..5;
# cols 25..29 = negW0[1..5]; col 30 = -1.
NWTAB = 31


def host_wtab(W, negW0) -> np.ndarray:
    row = np.zeros(NWTAB, dtype=np.int64)
    for p in range(1, NT):
        for k in range(1, NT):
            row[(p - 1) * 5 + (k - 1)] = W[p][k]
        row[25 + (p - 1)] = negW0[p]
    row[30] = 0xFFFFFFFF
    r32 = np.array([_i32(v) for v in row], dtype=np.int32)
    return np.tile(r32[None, :], (P, 1))


def build_nc(W, negW0, bs):
    assert bs == G * NQ
    i32 = mybir.dt.int32
    f32 = mybir.dt.float32

    nc = bass.Bass()
    dma_load = getattr(nc, DMA_LOAD_ENGINE)
    dma_store = getattr(nc, DMA_STORE_ENGINE)
    perf = nc.declare_dram_parameter("perf", [T, bs, 2], i32, isOutput=False)
    ids = nc.declare_dram_parameter("ids", [T, bs, 1], i32, isOutput=False)
    pred = nc.declare_dram_parameter("pred", [bs, 1], i32, isOutput=False)
    wtab = nc.declare_dram_parameter("wtab", [P, NWTAB], i32, isOutput=False)
    outp = nc.declare_dram_parameter("trust", [bs, 1], f32, isOutput=True)

    vec, gp, act = nc.vector, nc.gpsimd, nc.scalar
    AF = mybir.ActivationFunctionType

    with SplitDrainTileContext(nc) as tc:
        with tc.tile_pool(name="pers", bufs=1) as pers, \
             tc.tile_pool(name="dmain", bufs=NCH) as dmain, \
             tc.tile_pool(name="pool", bufs=2) as pool, \
             tc.tile_pool(name="actout", bufs=2) as actout, \
             tc.tile_pool(name="tree", bufs=2) as tree:
            # ---------------- per-core prep ----------------
            wt = pers.tile([P, NWTAB], i32, tag="wtab")
            dma_load.dma_start(out=wt[:, :], in_=wtab.rearrange("p n -> p n"))

            predt = pers.tile([P, NQ], i32, tag="predt")
            dma_load.dma_start(
                out=predt[:, :],
                in_=pred.rearrange("(g q) one -> g (q one)", g=G),
            )

            # presence masks p_b == p
            cpP = {}
            for p in range(1, NT):
                m = pers.tile([P, NQ], i32, tag=f"cpP{p}")
                vec.tensor_scalar(m[:, :], predt[:, :], p, None, AluOpType.is_equal)
                cpP[p] = m

            # Wfull[:, k-1, :] = W[p_b][k]  (k = 1..5)
            wfull = pers.tile([P, 5, NQ], i32, tag="wfull")
            vec.tensor_copy(
                wfull[:, :, :],
                wt[:, 0:5][:, :, None].broadcast_to([P, 5, NQ]),
            )
            for p in range(2, NT):
                vec.copy_predicated(
                    wfull[:, :, :],
                    cpP[p][:, None, :].broadcast_to([P, 5, NQ]),
                    wt[:, (p - 1) * 5 : (p - 1) * 5 + 5][:, :, None].broadcast_to(
                        [P, 5, NQ]
                    ),
                )

            # nw0[b] = ~W0[p_b]
            nw0 = pers.tile([P, NQ], i32, tag="nw0")
            vec.memset(nw0[:, :], 0)
            for p in range(1, NT):
                vec.copy_predicated(
                    nw0[:, :],
                    cpP[p][:, :],
                    wt[:, 25 + p - 1 : 26 + p - 1].broadcast_to([P, NQ]),
                )

            negone = wt[:, 30:31]  # [P, 1]

            # f32 bias tiles for ACT Square (bias must be an AP for non-Copy)
            actb = {}
            for k in (0, 4, 5):
                bt = pers.tile([P, 1], f32, tag=f"actb{k}")
                vec.memset(bt[:, :], float(-k))
                actb[k] = bt

            # ---------------- chunks over t ----------------
            states = None
            for ch in range(NCH):
                t0 = ch * TCC
                perf_t = dmain.tile([P, TH, NQ, 2], i32, tag="perf")
                ids_t = dmain.tile([P, TH, NQ], i32, tag="ids")
                dma_store.dma_start(
                    out=ids_t[:, :, :],
                    in_=ids[t0 : t0 + TCC].rearrange(
                        "th (g q) one -> g th (q one)", g=G
                    ),
                )
                dma_load.dma_start(
                    out=perf_t[:, :, :, :],
                    in_=perf[t0 : t0 + TCC].rearrange(
                        "th (g q) c -> g th q c", g=G
                    ),
                )

                # masks: m0, m4, m5 on ACT (Square + Relu); m2, m3 on DVE
                ids16 = actout.tile([P, TH, NQ], mybir.dt.int16, tag="ids16")
                act.activation(
                    ids16[:, :, :], ids_t[:, :, :], AF.Copy, bias=0.0, scale=1.0
                )
                msk = {}
                for k in (0, 4):
                    sq = actout.tile([P, TH, NQ], mybir.dt.int16, tag="sq")
                    mk = actout.tile([P, TH, NQ], mybir.dt.int16, tag=f"m{k}")
                    act.activation(
                        sq[:, :, :], ids_t[:, :, :], AF.Square,
                        bias=actb[k][:, :], scale=1.0,
                    )
                    act.activation(
                        mk[:, :, :], sq[:, :, :], AF.Relu, bias=1.0, scale=-1.0
                    )
                    msk[k] = mk
                for k in (2, 3, 5):
                    mk = actout.tile([P, TH, NQ], mybir.dt.int16, tag=f"m{k}")
                    vec.tensor_scalar(
                        mk[:, :, :], ids16[:, :, :], k, None, AluOpType.is_equal
                    )
                    msk[k] = mk

                # select w = Wrow[id]; default (id==1) seeded by init copy
                w = pool.tile([P, TH, NQ], i32, tag="w")
                act.activation(
                    w[:, :, :],
                    wfull[:, 0, :][:, None, :].broadcast_to([P, TH, NQ]),
                    AF.Copy, bias=0.0, scale=1.0,
                )
                vec.copy_predicated(
                    w[:, :, :], msk[0][:, :, :],
                    negone[:, :, None].broadcast_to([P, TH, NQ]),
                )
                for k in (2, 3, 4, 5):
                    vec.copy_predicated(
                        w[:, :, :], msk[k][:, :, :],
                        wfull[:, k - 1, :][:, None, :].broadcast_to([P, TH, NQ]),
                    )

                # gates: A0 = w & -p0 == w*p0 (p0 in {0,1}, Pool int mult);
                # V0 = w | (p1-1) with s1 = p1-1 from ACT (bitwise-or is DVE-only)
                A0 = pool.tile([P, TH, NQ], i32, tag="A0")
                vec.scalar_tensor_tensor(
                    A0[:, :, :], w[:, :, :], 1, perf_t[:, :, :, 0],
                    AluOpType.mult, AluOpType.mult,
                )
                s1 = actout.tile([P, TH, NQ], i32, tag="s1")
                act.activation(
                    s1[:, :, :], perf_t[:, :, :, 1], AF.Copy, bias=-1.0, scale=1.0
                )
                V0 = pool.tile([P, TH, NQ], i32, tag="V0")
                vec.tensor_tensor(
                    V0[:, :, :], w[:, :, :], s1[:, :, :], AluOpType.bitwise_or
                )

                # in-partition tree over th: 8 -> 4 -> 2 -> 1
                A, V = A0, V0
                nt = TH
                lvl = 0
                while nt > 1:
                    nt //= 2
                    lvl += 1
                    eng = vec
                    An = tree.tile([P, nt, NQ], i32, tag=f"A{lvl}")
                    Vn = tree.tile([P, nt, NQ], i32, tag=f"V{lvl}")
                    AL, AR = A[:, 0::2, :], A[:, 1::2, :]
                    VL, VR = V[:, 0::2, :], V[:, 1::2, :]
                    eng.tensor_tensor(Vn[:, :, :], VL, AR, AluOpType.bitwise_or)
                    eng.tensor_tensor(
                        Vn[:, :, :], Vn[:, :, :], VR, AluOpType.bitwise_and
                    )
                    eng.tensor_tensor(An[:, :, :], AL, AR, AluOpType.bitwise_or)
                    A, V = An, Vn

                A4, V4 = A, V

                # fold into running cross-chunk state (t order)
                if ch == 0:
                    states = [A4, V4]
                else:
                    Ar, Vr = states
                    Vn = tree.tile([P, 1, NQ], i32, tag=f"Vc{ch}")
                    vec.tensor_tensor(
                        Vn[:, :, :], Vr[:, :, :], A4[:, :, :], AluOpType.bitwise_or
                    )
                    vec.tensor_tensor(
                        Vn[:, :, :], Vn[:, :, :], V4[:, :, :], AluOpType.bitwise_and
                    )
                    if ch < NCH - 1:
                        An = tree.tile([P, 1, NQ], i32, tag=f"Ac{ch}")
                        vec.tensor_tensor(
                            An[:, :, :], Ar[:, :, :], A4[:, :, :],
                            AluOpType.bitwise_or,
                        )
                    else:
                        An = A4
                    states = [An, Vn]
            A, V = states

            # ---------------- finalize ----------------
            x = tree.tile([P, NQ], i32, tag="fin")
            vec.tensor_tensor(x[:, :], V[:, 0, :], nw0[:, :], AluOpType.bitwise_and)
            vec.tensor_scalar(x[:, :], x[:, :], 0, None, AluOpType.is_equal)
            of = tree.tile([P, NQ], f32, tag="of")
            vec.tensor_copy(of[:, :], x[:, :])
            dma_store.dma_start(
                out=outp.rearrange("(g q) one -> g (q one)", g=G), in_=of[:, :]
            )

    split_multi_waits(nc)
    return nc


_CACHE = {}


def _get_nc(key, W, negW0, bs):
    if key not in _CACHE:
        _CACHE[key] = build_nc(W, negW0, bs)
    return _CACHE[key]


def build_in_maps(perf, ids, pred, wtab_np, bs):
    in_maps = []
    for c in range(NCORES):
        sl = slice(c * bs, (c + 1) * bs)
        in_maps.append(
            {
                "perf": perf[:, sl, :],
                "ids": ids[:, sl, :],
                "pred": pred[sl, :],
                "wtab": wtab_np,
            }
        )
    return in_maps


def kernel(inptasksperf, tasksobsids, taskspredids, obsMatrix):
    perf = np.ascontiguousarray(np.asarray(inptasksperf, dtype=np.int32))
    ids = np.ascontiguousarray(np.asarray(tasksobsids, dtype=np.int32))
    pred = np.ascontiguousarray(np.asarray(taskspredids, dtype=np.int32))
    M = np.asarray(obsMatrix, dtype=np.float32)

    W, negW0 = host_tables(M)
    wtab_np = host_wtab(W, negW0)
    bs = B // NCORES
    key = (W.tobytes(), negW0.tobytes(), bs)
    nc = _get_nc(key, W, negW0, bs)

    in_maps = build_in_maps(perf, ids, pred, wtab_np, bs)
    res = run_bass_kernel_spmd(nc, in_maps, list(range(NCORES)))
    out = np.concatenate([res.results[c]["trust"] for c in range(NCORES)], axis=0)
    return out.astype(np.float32)
